# revision 55
# baseline (speedup 1.0000x reference)
"""MultiHeadSelfAttention Trainium2 kernel (8 NeuronCores, SPMD).

Problem: x[2,2048,1024], H=16 heads, hd=64.  out = softmax(QK^T/8)V + x.

Sharding (tensor-parallel over heads x data-parallel over batch):
  core c (0..7): batch b = c//4, head group g = c%4 -> heads [4g, 4g+4),
  i.e. output columns [256g, 256g+256) of batch b.  No collectives.

Per-core dataflow (v3 -- rebuilt from the 209.8us baseline's trace):
  - K bias dropped entirely: softmax over k is invariant to the per-query
    constant q.bk, so scores ~ q.(x Wk)^T.
  - input pre-swizzled on host to the exact SBUF layout [128, 6, 8, 512]
    (chunks: [wq|wk], [wv|-], x tokens per 512) so each chunk DMA moves
    8KB-contiguous runs per partition; chunk-staged emission lets the
    first score matmul run ~12us in instead of ~40us.
  - Q^T/K^T [128,2,2048] bf16: head pair m=(2m,2m+1) on partition halves.
  - scores^T via 64-row matmuls with tile_position=(0,0)/(64,0): the two
    heads' MMs run concurrently on disjoint PE row groups.
  - 64-row MMs do not register as PE-HAM activity, so a zero-valued
    128-row N=64 matmul (accumulating +0 onto the live ring slice) is
    issued every other ACT chunk to keep the PE clock at 2.4 GHz.
  - exp on ACT in 1536-elem chunks through a 6-bank PSUM ring (2 bufs x
    [128,1536]): amortizes the ~182-cycle per-instruction ACT overhead.
    exps written as fp8e4 (values in [0,~8], far below the 240 cap).
  - AV in fp8 DoubleRow (K=256 per MM): V [128,kt,4*68] fp8 with a ones
    column per head (68-col stride keeps the DoubleRow AP 16B-aligned);
    the ones column makes the AV matmul emit sum(exp) for free.
  - transpose of the [65,512] AV output via plain matmul against a bf16
    identity (N=65 stream) instead of transpose-mode.
  - normalize+residual fused in one DVE scalar_tensor_tensor:
    out = (outT^T * recip(sumexp)) + x.  Reciprocals batched 4-wide.
  - emission hand-woven: score-MM chunks of block i interleave with
    AV/output work of block i-1 (and V/proj work early) so PE gaps stay
    inside HAM's 3.4us window and ACT stays saturated.
"""

import ml_dtypes
import numpy as np

B, S, D, H = 2, 2048, 1024, 16
HD = 64
NCORES = 8
GH = 4            # heads per core
GD = GH * HD      # 256 output columns per core
P = 128
DT = D // P       # 8 contraction tiles
KT = S // P       # 16 k-tiles
VC = 68           # per-head V cols (64 data + 1 ones + 3 pad for 16B stride)
NQT = S // P      # 16 query tiles of 128
NU = KT * 2 * 2   # 64 scores-MM units per (head-pair, 1024-q) block
NCH = 6           # xw chunks: [wq|wk], [wv|-], 4x 512 tokens

_CACHE = {}
TRACE = False
LAST_RESULTS = None
# Row-tiled 64-row score MMs measured fully serial (no tile concurrency) AND
# invisible to the PE-HAM clock gate -> ~40us of 1.2GHz throttling.  The
# zero-padded 128-row form costs the same PE cycles and keeps the clock warm.
USE_ROWTILE = False
USE_FP8AV = True


def _build_nc():
    import concourse.bass as bass
    import concourse.mybir as mybir
    import concourse.tile as tile
    from concourse import bacc
    from concourse.masks import make_identity

    f32 = mybir.dt.float32
    bf16 = mybir.dt.bfloat16
    fp8 = mybir.dt.float8e4
    EXP = mybir.ActivationFunctionType.Exp
    ADD = mybir.AluOpType.add
    MULT = mybir.AluOpType.mult
    DR = mybir.MatmulPerfMode.DoubleRow

    av_dt = fp8 if USE_FP8AV else bf16

    nc = bacc.Bacc("TRN2")

    wa_d = nc.dram_tensor("wa", [P, DT * 512], fp8, kind="ExternalInput")
    wb_d = nc.dram_tensor("wb", [P, DT * GD], fp8, kind="ExternalInput")
    x_d = nc.dram_tensor("xt", [P, 4 * DT * 512], fp8, kind="ExternalInput")
    bq_d = nc.dram_tensor("bq", [GD], f32, kind="ExternalInput")
    bv_d = nc.dram_tensor("bv", [GD], f32, kind="ExternalInput")
    xres_d = nc.dram_tensor("xres", [S, GD], bf16, kind="ExternalInput")
    out_d = nc.dram_tensor("out", [S, GD], f32, kind="ExternalOutput")

    with tile.TileContext(nc) as tc:
        with (
            tc.tile_pool(name="persist", bufs=1) as persist,
            tc.tile_pool(name="expp", bufs=2) as expp,
            tc.tile_pool(name="work", bufs=3) as work,
            tc.tile_pool(name="psum", bufs=2, space="PSUM") as psum,
        ):
            # ---- input DMAs.  The DMA rings round-robin all pending
            # transfers, so the chunks the first matmuls need are issued
            # first and later chunks are gated behind DVE markers (a real
            # data dependency) so they don't steal startup bandwidth. ----
            wa_sb = persist.tile([P, DT, 512], fp8, tag="wa_sb")   # wq|wk
            nc.sync.dma_start(wa_sb, wa_d.rearrange("p (dt s) -> p dt s", dt=DT))
            wb_sb = persist.tile([P, DT, GD], fp8, tag="wb_sb")    # wv
            x_sb = persist.tile([P, 4, DT, 512], fp8, tag="x_sb")
            x_r = x_d.rearrange("p (c dt s) -> p c dt s", c=4, dt=DT)
            nc.scalar.dma_start(x_sb[:, 0], x_r[:, 0])    # x tokens 0-511

            wq_sb = wa_sb[:, :, 0:GD]
            wk_sb = wa_sb[:, :, GD:2 * GD]
            wv_sb = wb_sb

            def xt(c):
                """x^T tokens [c*512, (c+1)*512) as [P, DT, 512]."""
                return x_sb[:, c]

            bq_sb = persist.tile([P, 2], f32, tag="bq_sb")
            nc.sync.dma_start(bq_sb, bq_d.rearrange("(m p) -> p m", p=P))
            bv_bc = persist.tile([P, GD], f32, tag="bv_bc")
            bv_ap = bass.AP(
                tensor=bv_d[:].tensor, offset=bv_d[:].offset,
                ap=[[0, P]] + list(bv_d[:].ap),
            )
            nc.gpsimd.dma_start(out=bv_bc, in_=bv_ap)

            xres_sb = persist.tile([P, NQT, GD], bf16, tag="xres_sb")
            xres_r = xres_d.rearrange("(t p) c -> p t c", p=P)

            def gated_dma(engine, dst, src):
                # DVE write to the destination's first element forces the
                # DMA to wait for the marker's queue position
                nc.vector.memset(dst[(slice(None),) + (0,) * (len(dst.shape) - 2)
                                     + (slice(0, 1),)], 0.0)
                engine.dma_start(dst, src)

            x23_gate = [
                lambda: gated_dma(nc.sync, x_sb[:, 2], x_r[:, 2]),
                lambda: gated_dma(nc.sync, x_sb[:, 3], x_r[:, 3]),
            ]
            xres_gate = [
                lambda: gated_dma(nc.sync, xres_sb[:, 0:8, :], xres_r[:, 0:8, :]),
                lambda: gated_dma(nc.sync, xres_sb[:, 8:16, :], xres_r[:, 8:16, :]),
            ]

            identity = persist.tile([P, P], bf16, tag="identity")
            make_identity(nc, identity)
            zero_sb = persist.tile([P, 512], bf16, tag="zero_sb")
            nc.vector.memset(zero_sb, 0.0)

            # pre-warm the PE while the first DMAs land: junk matmuls
            # take the HAM clock gate to 8/8 before the real work starts
            # (sized to span the DMA wait so no idle window re-throttles)
            warm_ps = psum.tile([HD, 512], f32, tag="flex", name="warm_ps")
            for _ in range(22):
                nc.tensor.matmul(
                    warm_ps, lhsT=zero_sb[:, 0:HD], rhs=zero_sb,
                    start=True, stop=True)

            # x tokens 512-1023 is needed a few us after the first
            # projections: release its DMA from the gpsimd queue behind a
            # ~2us memset shim so wa/x0/wv get the startup bandwidth alone
            v_sb = persist.tile([P, KT, GH * VC], av_dt, tag="v_sb")
            nc.sync.dma_start(
                wb_sb, wb_d.rearrange("p (dt s) -> p dt s", dt=DT))
            nc.gpsimd.memset(v_sb[:, 0:8, :], 0.0)
            nc.gpsimd.dma_start(x_sb[:, 1], x_r[:, 1])    # x 512-1023

            qT_sb = persist.tile([P, 2, S], bf16, tag="qT_sb")
            if USE_ROWTILE:
                kT_sb = persist.tile([P, 2, S], bf16, tag="kT_sb")

                def kzero_unit(m, c):
                    pass
            else:
                kT_sb = persist.tile([P, GH, S], bf16, tag="kT_sb")
                kq = kT_sb.rearrange("p (m two) s -> p m two s", two=2)

                def kzero_unit(m, c):
                    # zero the dead partition halves of head-pair m's K slots
                    # for 512 tokens -- emitted just-in-time so the bulk
                    # memset doesn't sit ahead of the first evacuations in
                    # the DVE queue
                    sl = slice(c * 512, (c + 1) * 512)
                    nc.vector.memset(kq[HD:, m, 0, sl], 0.0)
                    nc.vector.memset(kq[:HD, m, 1, sl], 0.0)

            def ones_unit():
                nc.vector.memset(
                    v_sb.rearrange(
                        "p t (h c) -> p t h c", c=VC)[:, :, :, HD:HD + 1],
                    1.0)
            out_sb = persist.tile([P, NQT, GD], f32, tag="out_sb")

            # pre-observe DMA'd constants on DVE so downstream DVE ops don't
            # carry a DMA wait alongside a PE wait
            sink = persist.tile([P, 4], f32, tag="sink")
            nc.vector.tensor_copy(sink[:, 0:1], bv_bc[:, 0:1])
            nc.vector.tensor_copy(sink[:, 1:2], bq_sb[:, 0:1])
            xres_sink = [
                lambda: nc.vector.tensor_copy(sink[:, 2:3], xres_sb[:, 0, 0:1]),
                lambda: nc.vector.tensor_copy(sink[:, 3:4], xres_sb[:, 8, 0:1]),
            ]

            # ---- projections (fp8 DoubleRow: 256-deep contraction/MM) ----
            def proj_qk_unit(which, m, c):
                w = wq_sb if which == "q" else wk_sb
                ps = psum.tile([P, 512], f32, tag="flex", name="ps_proj")
                xc = xt(c)
                for dp in range(DT // 2):
                    nc.tensor.matmul(
                        ps, lhsT=w[:, 2 * dp:2 * dp + 2, m * P:(m + 1) * P],
                        rhs=xc[:, 2 * dp:2 * dp + 2, :],
                        start=(dp == 0), stop=(dp == DT // 2 - 1),
                        perf_mode=DR,
                    )
                sl = slice(c * 512, (c + 1) * 512)
                if which == "q":
                    nc.vector.tensor_scalar_add(
                        qT_sb[:, m, sl], ps, bq_sb[:, m:m + 1])
                elif USE_ROWTILE:
                    nc.vector.tensor_copy(kT_sb[:, m, sl], ps)
                else:
                    nc.vector.tensor_copy(kT_sb[:HD, 2 * m, sl], ps[:HD])
                    nc.vector.tensor_copy(kT_sb[HD:, 2 * m + 1, sl], ps[HD:])

            def v_unit(tt):
                ps = psum.tile([P, GD], f32, tag="flex", name="ps_v")
                xc = xt(tt // 4)
                o = (tt % 4) * P
                for dp in range(DT // 2):
                    nc.tensor.matmul(
                        ps, lhsT=xc[:, 2 * dp:2 * dp + 2, o:o + P],
                        rhs=wv_sb[:, 2 * dp:2 * dp + 2, :],
                        start=(dp == 0), stop=(dp == DT // 2 - 1),
                        perf_mode=DR,
                    )
                nc.vector.tensor_tensor(
                    v_sb[:, tt, :].rearrange(
                        "p (h c) -> p h c", c=VC)[:, :, :HD],
                    ps.rearrange("p (h c) -> p h c", c=HD),
                    bv_bc.rearrange("p (h c) -> p h c", c=HD),
                    ADD)

            # ---- attention ----
            def av_mm_unit(m, qb, par, qh, exps, t0=0, t1=None, pso=None):
                h = 2 * m + par
                if pso is None:
                    pso = psum.tile([65, 512], f32, tag="flex", name="pso")
                ex = exps.rearrange(
                    "p (qh kt par q) -> p qh kt par q", kt=KT, qh=2, par=2)
                n = KT // 2 if USE_FP8AV else KT
                rng = range(n) if t1 is None else range(t0, t1)
                for t in rng:
                    first = t == 0
                    last = t == n - 1
                    if USE_FP8AV:
                        nc.tensor.matmul(
                            pso,
                            lhsT=v_sb[:, 2 * t:2 * t + 2, h * VC:h * VC + 65],
                            rhs=ex[:, qh, 2 * t:2 * t + 2, par, :],
                            start=first, stop=last, perf_mode=DR)
                    else:
                        nc.tensor.matmul(
                            pso,
                            lhsT=v_sb[:, t, h * VC:h * VC + 65],
                            rhs=ex[:, qh, t, par, :],
                            start=first, stop=last)
                return pso

            def av_out_unit(m, qb, par, qh, pso, dma_qt, act_copy=False):
                h = 2 * m + par
                oT = work.tile([65, 512], bf16, tag="oT", name="oT")
                if act_copy:
                    # final-block output chain: the exp stream is done, so
                    # the idle ACT engine does the PSUM evacuation instead
                    # of the busy DVE
                    nc.scalar.copy(oT, pso)
                else:
                    nc.vector.tensor_copy(oT, pso)
                pst = psum.tile([P, 4 * 65], f32, tag="flex", name="pst")
                for i in range(4):
                    nc.tensor.matmul(
                        pst[:, i * 65:(i + 1) * 65],
                        lhsT=oT[:, i * 128:(i + 1) * 128],
                        rhs=identity[:65, :65],
                        start=True, stop=True)
                r = work.tile([P, 4], f32, tag="r", name="r")
                nc.vector.reciprocal(
                    r.rearrange("p (i c) -> p i c", c=1),
                    pst.rearrange("p (i c) -> p i c", c=65)[:, :, 64:65])
                # out-DMAs spread across engine queues (issuance costs
                # ~0.6us per DMA on one sequencer); scalar only at the very
                # tail when the exp stream is finished
                engs = [nc.sync, nc.gpsimd, nc.sync, nc.gpsimd]
                if m == 1 and qb == 1 and qh == 1:
                    engs = [nc.sync, nc.gpsimd, nc.scalar, nc.sync]
                for i in range(4):
                    qt = qb * 8 + qh * 4 + i
                    nc.vector.scalar_tensor_tensor(
                        out=out_sb[:, qt, h * HD:(h + 1) * HD],
                        in0=pst[:, i * 65:i * 65 + 64],
                        scalar=r[:, i:i + 1],
                        in1=xres_sb[:, qt, h * HD:(h + 1) * HD],
                        op0=MULT, op1=ADD)
                    if dma_qt:
                        engs[i].dma_start(
                            out_d[qt * P:(qt + 1) * P, :], out_sb[:, qt, :])

            def block_scores(m, qb, exps, fill, pre):
                """scores^T + exp for head pair m, 1024-wide q block qb.

                unit u = (qh, kt, par):   exps flat = (qh kt par q)
                fill[i]: emitted after the i-th ACT chunk (None = skip).
                pre[u]: callables emitted before unit u's matmul.
                """
                fi = 0
                ci = 0
                ring_t = None
                chunk_start = 0
                for u in range(NU):
                    qh, kt, par = u // 32, (u // 2) % KT, u % 2
                    for fn in pre.get(u, ()):
                        fn()
                    # runt chunk (1 unit) first: the next block's exp stream
                    # restarts one short matmul after the boundary
                    j = 0 if u == 0 else (u - 1) % 3
                    if j == 0:
                        ring_t = psum.tile(
                            [P, 1536], f32, tag="ring", name="ring")
                        chunk_start = u
                    q0 = qb * 1024 + qh * 512
                    if USE_ROWTILE:
                        rows = slice(par * HD, (par + 1) * HD)
                        nc.tensor.matmul(
                            ring_t[:, j * 512:(j + 1) * 512],
                            lhsT=kT_sb[rows, m, kt * P:(kt + 1) * P],
                            rhs=qT_sb[rows, m, q0:q0 + 512],
                            start=True, stop=True,
                            tile_position=(par * HD, 0))
                    else:
                        nc.tensor.matmul(
                            ring_t[:, j * 512:(j + 1) * 512],
                            lhsT=kT_sb[:, 2 * m + par, kt * P:(kt + 1) * P],
                            rhs=qT_sb[:, m, q0:q0 + 512],
                            start=True, stop=True)
                    if j == 2 or u == 0 or u == NU - 1:
                        w = (u - chunk_start + 1) * 512
                        dst = exps[:, chunk_start * 512:chunk_start * 512 + w]
                        if USE_FP8AV and ci % 4 == 2:
                            # fast-exp on DVE: uint8(round(s*log2e + 8*bias))
                            # IS fp8e4 of exp2(s*log2e) up to the linear-
                            # mantissa approx (<6%, and the ones-column
                            # denominator sums the same stored values).
                            # Offloads ~25% of exp from the saturated ACT.
                            nc.vector.tensor_scalar(
                                out=dst.bitcast(mybir.dt.uint8),
                                in0=ring_t[:, :w],
                                scalar1=0.125 * 1.4426950408889634,
                                scalar2=56.0,
                                op0=MULT, op1=ADD)
                        else:
                            nc.scalar.activation(
                                out=dst, in_=ring_t[:, :w],
                                func=EXP, scale=0.125)
                        ci += 1
                        if fi < len(fill):
                            if fill[fi] is not None:
                                fill[fi]()
                            fi += 1
                for fn in fill[fi:]:
                    if fn is not None:
                        fn()

            blocks = [(0, 0), (0, 1), (1, 0), (1, 1)]
            exps_t = {}

            def av_fill(b, qhs=(0, 1), mm_first=False):
                # AV matmuls split into two halves so no single fill item
                # exceeds the per-ACT-chunk PE slack (~0.8us)
                m, qb = blocks[b]
                state = {}
                items = []
                n = KT // 2 if USE_FP8AV else KT
                for qh in qhs:
                    pair = []
                    for par in (0, 1):
                        def mk_mm_a(par=par, qh=qh):
                            state[(par, qh)] = av_mm_unit(
                                m, qb, par, qh, exps_t[b], 0, n // 2)

                        def mk_mm_b(par=par, qh=qh):
                            av_mm_unit(m, qb, par, qh, exps_t[b], n // 2, n,
                                       pso=state[(par, qh)])

                        def mk_out(par=par, qh=qh):
                            av_out_unit(m, qb, par, qh, state[(par, qh)],
                                        dma_qt=(m == 1 and par == 1))
                        pair += [mk_mm_a, mk_mm_b, mk_out]
                    if mm_first:
                        # both AV matmul groups back-to-back, then the two
                        # output chains (shorter critical path at the tail)
                        pair = [pair[0], pair[1], pair[3], pair[4],
                                pair[2], pair[5]]
                    items += pair
                return items

            def av_stream(b, qh):
                """Final-block AV: accumulate in quarters as the exps land,
                so only the last quarter and the output chain trail the
                exp stream."""
                m, qb = blocks[b]
                n = KT // 2 if USE_FP8AV else KT
                state = {}
                items = []
                for j in range(4):
                    for par in (0, 1):
                        def mk(j=j, par=par):
                            t0, t1 = j * n // 4, (j + 1) * n // 4
                            state[par] = av_mm_unit(
                                m, qb, par, qh, exps_t[b], t0, t1,
                                pso=state.get(par))
                        items.append(mk)
                for par in (0, 1):
                    def mk_out(par=par):
                        av_out_unit(m, qb, par, qh, state[par],
                                    dma_qt=(m == 1 and par == 1),
                                    act_copy=True)
                    items.append(mk_out)
                return items

            def spread(items, slots=22):
                out = []
                for i, it in enumerate(items):
                    out.append(it)
                    if i % 3 == 2 and len(out) + (len(items) - i - 1) < slots:
                        out.append(None)
                return out

            def interleave(a, b):
                out = []
                for x, y in zip(a, b):
                    out += [x, y]
                out += a[len(b):] + b[len(a):]
                return out

            # prologue: only what the first score matmuls need
            kzero_unit(0, 0)
            proj_qk_unit("k", 0, 0)
            proj_qk_unit("q", 0, 0)

            for b, (m, qb) in enumerate(blocks):
                exps_t[b] = expp.tile(
                    [P, NU * 512], av_dt, tag="exps", name="exps")
                pre = {}
                if b == 0:
                    # K M0 chunks staged at the kt boundaries that need them;
                    # Q M0 chunk 1 before the qh=1 half; gated DMAs released
                    # once the startup-critical transfers are done
                    # pre keys sit on chunk starts (u = 1+3k) so a staged
                    # projection never lands inside a chunk's matmul group
                    pre = {
                        4: [lambda: proj_qk_unit("k", 0, 1),
                            x23_gate[0], x23_gate[1], ones_unit],
                        7: [lambda: kzero_unit(0, 1)],
                        10: [lambda: proj_qk_unit("k", 0, 2)],
                        16: [lambda: kzero_unit(0, 2),
                             xres_gate[0], xres_sink[0]],
                        19: [lambda: proj_qk_unit("k", 0, 3)],
                        22: [lambda: kzero_unit(0, 3),
                             xres_gate[1], xres_sink[1]],
                        31: [lambda: proj_qk_unit("q", 0, 1)],
                    }
                    # V units, then Q chunk 2 (needed by block1's qh=0), then
                    # block0's own qh=0 AV work -- its exps are complete by
                    # chunk 11, and block1 has no room for it (scores + M1
                    # projections + its own AV would oversubscribe the PE)
                    fill = [
                        (lambda tt=tt: v_unit(tt)) for tt in range(KT)
                    ] + [
                        lambda: proj_qk_unit("q", 0, 2),
                    ] + av_fill(0, qhs=(0,))
                elif b == 1:
                    def qc3_kz(_=None):
                        proj_qk_unit("q", 0, 3)
                        kzero_unit(1, 2)
                        kzero_unit(1, 3)

                    def kz01(_=None):
                        kzero_unit(1, 0)
                        kzero_unit(1, 1)
                    proj1 = [
                        (lambda c=c: proj_qk_unit("k", 1, c)) for c in range(4)
                    ] + [
                        (lambda c=c: proj_qk_unit("q", 1, c)) for c in range(4)
                    ]
                    fill = spread([kz01, qc3_kz] + interleave(
                        proj1, av_fill(0, qhs=(1,))))
                elif b == 2:
                    fill = spread(av_fill(1))
                else:
                    # previous block's AV, this block's qh=0 AV once its
                    # exps complete (chunk 11), then the qh=1 AV streamed
                    # in quarters behind the exp stream
                    st = av_stream(3, 1)
                    fill = av_fill(2) + av_fill(3, qhs=(0,)) + st[:4]
                    tail_items = st[4:]
                block_scores(m, qb, exps_t[b], fill, pre)

            for it in tail_items:
                it()

    nc.finalize()
    return nc


def _get_nc():
    if "nc" not in _CACHE:
        _CACHE["nc"] = _build_nc()
    return _CACHE["nc"]


def _swizzle(a):
    """[1024, n] -> [128, 8, n] with row d at (d % 128, d // 128)."""
    return np.ascontiguousarray(
        a.reshape(DT, P, a.shape[1]).transpose(1, 0, 2))


FP8 = ml_dtypes.float8_e4m3  # IEEE e4m3 (bias 7, max 240) = TRN FP8_EXP4


def kernel(x, Wq, bq, Wk, bk, Wv, bv):
    global LAST_RESULTS
    from concourse.bass_utils import run_bass_kernel_spmd

    x = np.asarray(x, dtype=np.float32)
    Wq, Wk, Wv = (np.asarray(a, dtype=np.float32) for a in (Wq, Wk, Wv))
    bq, bv = (np.asarray(a, dtype=np.float32) for a in (bq, bv))

    xTs = [np.ascontiguousarray(x[b].T) for b in range(B)]
    xt8 = []
    for b in range(B):
        xc = np.empty((P, 4, DT, 512), dtype=FP8)
        for ch in range(4):
            xc[:, ch] = _swizzle(
                xTs[b][:, ch * 512:(ch + 1) * 512]).astype(FP8)
        xt8.append(np.ascontiguousarray(xc.reshape(P, 4 * DT * 512)))
    in_maps = []
    for c in range(NCORES):
        b, g = c // 4, c % 4
        cols = slice(GD * g, GD * (g + 1))
        wa = np.empty((P, DT, 512), dtype=FP8)
        wa[:, :, 0:GD] = _swizzle(Wq[:, cols]).astype(FP8)
        wa[:, :, GD:2 * GD] = _swizzle(Wk[:, cols]).astype(FP8)
        in_maps.append({
            "wa": np.ascontiguousarray(wa.reshape(P, DT * 512)),
            "wb": np.ascontiguousarray(
                _swizzle(Wv[:, cols]).astype(FP8).reshape(P, DT * GD)),
            "xt": xt8[b],
            "bq": np.ascontiguousarray(bq[cols]),
            "bv": np.ascontiguousarray(bv[cols]),
            "xres": np.ascontiguousarray(
                x[b][:, cols].astype(ml_dtypes.bfloat16)),
        })

    nc = _get_nc()
    res = run_bass_kernel_spmd(
        nc, in_maps, core_ids=list(range(NCORES)), trace=TRACE,
    )
    LAST_RESULTS = res

    full = np.empty((B, S, D), dtype=np.float32)
    for c in range(NCORES):
        b, g = c // 4, c % 4
        full[b, :, GD * g:GD * (g + 1)] = res.results[c]["out"]
    return full


# revision 56
# speedup vs baseline: 1.0089x; 1.0089x over previous
"""MultiHeadSelfAttention Trainium2 kernel (8 NeuronCores, SPMD).

Problem: x[2,2048,1024], H=16 heads, hd=64.  out = softmax(QK^T/8)V + x.

Sharding (tensor-parallel over heads x data-parallel over batch):
  core c (0..7): batch b = c//4, head group g = c%4 -> heads [4g, 4g+4),
  i.e. output columns [256g, 256g+256) of batch b.  No collectives.

Per-core dataflow (v3 -- rebuilt from the 209.8us baseline's trace):
  - K bias dropped entirely: softmax over k is invariant to the per-query
    constant q.bk, so scores ~ q.(x Wk)^T.
  - input pre-swizzled on host to the exact SBUF layout [128, 6, 8, 512]
    (chunks: [wq|wk], [wv|-], x tokens per 512) so each chunk DMA moves
    8KB-contiguous runs per partition; chunk-staged emission lets the
    first score matmul run ~12us in instead of ~40us.
  - Q^T/K^T [128,2,2048] bf16: head pair m=(2m,2m+1) on partition halves.
  - scores^T via 64-row matmuls with tile_position=(0,0)/(64,0): the two
    heads' MMs run concurrently on disjoint PE row groups.
  - 64-row MMs do not register as PE-HAM activity, so a zero-valued
    128-row N=64 matmul (accumulating +0 onto the live ring slice) is
    issued every other ACT chunk to keep the PE clock at 2.4 GHz.
  - exp on ACT in 1536-elem chunks through a 6-bank PSUM ring (2 bufs x
    [128,1536]): amortizes the ~182-cycle per-instruction ACT overhead.
    exps written as fp8e4 (values in [0,~8], far below the 240 cap).
  - AV in fp8 DoubleRow (K=256 per MM): V [128,kt,4*68] fp8 with a ones
    column per head (68-col stride keeps the DoubleRow AP 16B-aligned);
    the ones column makes the AV matmul emit sum(exp) for free.
  - transpose of the [65,512] AV output via plain matmul against a bf16
    identity (N=65 stream) instead of transpose-mode.
  - normalize+residual fused in one DVE scalar_tensor_tensor:
    out = (outT^T * recip(sumexp)) + x.  Reciprocals batched 4-wide.
  - emission hand-woven: score-MM chunks of block i interleave with
    AV/output work of block i-1 (and V/proj work early) so PE gaps stay
    inside HAM's 3.4us window and ACT stays saturated.
"""

import ml_dtypes
import numpy as np

B, S, D, H = 2, 2048, 1024, 16
HD = 64
NCORES = 8
GH = 4            # heads per core
GD = GH * HD      # 256 output columns per core
P = 128
DT = D // P       # 8 contraction tiles
KT = S // P       # 16 k-tiles
VC = 68           # per-head V cols (64 data + 1 ones + 3 pad for 16B stride)
NQT = S // P      # 16 query tiles of 128
NU = KT * 2 * 2   # 64 scores-MM units per (head-pair, 1024-q) block
NCH = 6           # xw chunks: [wq|wk], [wv|-], 4x 512 tokens

_CACHE = {}
TRACE = False
LAST_RESULTS = None
# Row-tiled 64-row score MMs measured fully serial (no tile concurrency) AND
# invisible to the PE-HAM clock gate -> ~40us of 1.2GHz throttling.  The
# zero-padded 128-row form costs the same PE cycles and keeps the clock warm.
USE_ROWTILE = False
USE_FP8AV = True


def _build_nc():
    import concourse.bass as bass
    import concourse.mybir as mybir
    import concourse.tile as tile
    from concourse import bacc
    from concourse.masks import make_identity

    f32 = mybir.dt.float32
    bf16 = mybir.dt.bfloat16
    fp8 = mybir.dt.float8e4
    EXP = mybir.ActivationFunctionType.Exp
    ADD = mybir.AluOpType.add
    MULT = mybir.AluOpType.mult
    DR = mybir.MatmulPerfMode.DoubleRow

    av_dt = fp8 if USE_FP8AV else bf16

    nc = bacc.Bacc("TRN2")

    wa_d = nc.dram_tensor("wa", [P, DT * 512], fp8, kind="ExternalInput")
    wb_d = nc.dram_tensor("wb", [P, DT * GD], fp8, kind="ExternalInput")
    x_d = nc.dram_tensor("xt", [P, 4 * DT * 512], fp8, kind="ExternalInput")
    bq_d = nc.dram_tensor("bq", [GD], f32, kind="ExternalInput")
    bv_d = nc.dram_tensor("bv", [GD], f32, kind="ExternalInput")
    xres_d = nc.dram_tensor("xres", [S, GD], bf16, kind="ExternalInput")
    out_d = nc.dram_tensor("out", [S, GD], f32, kind="ExternalOutput")

    with tile.TileContext(nc) as tc:
        with (
            tc.tile_pool(name="persist", bufs=1) as persist,
            tc.tile_pool(name="expp", bufs=2) as expp,
            tc.tile_pool(name="work", bufs=3) as work,
            tc.tile_pool(name="psum", bufs=2, space="PSUM") as psum,
        ):
            # ---- input DMAs.  The DMA rings round-robin all pending
            # transfers, so the chunks the first matmuls need are issued
            # first and later chunks are gated behind DVE markers (a real
            # data dependency) so they don't steal startup bandwidth. ----
            wa_sb = persist.tile([P, DT, 512], fp8, tag="wa_sb")   # wq|wk
            nc.sync.dma_start(wa_sb, wa_d.rearrange("p (dt s) -> p dt s", dt=DT))
            wb_sb = persist.tile([P, DT, GD], fp8, tag="wb_sb")    # wv
            x_sb = persist.tile([P, 4, DT, 512], fp8, tag="x_sb")
            x_r = x_d.rearrange("p (c dt s) -> p c dt s", c=4, dt=DT)
            nc.scalar.dma_start(x_sb[:, 0], x_r[:, 0])    # x tokens 0-511

            wq_sb = wa_sb[:, :, 0:GD]
            wk_sb = wa_sb[:, :, GD:2 * GD]
            wv_sb = wb_sb

            def xt(c):
                """x^T tokens [c*512, (c+1)*512) as [P, DT, 512]."""
                return x_sb[:, c]

            bq_sb = persist.tile([P, 2], f32, tag="bq_sb")
            nc.sync.dma_start(bq_sb, bq_d.rearrange("(m p) -> p m", p=P))
            bv_bc = persist.tile([P, GD], f32, tag="bv_bc")
            bv_ap = bass.AP(
                tensor=bv_d[:].tensor, offset=bv_d[:].offset,
                ap=[[0, P]] + list(bv_d[:].ap),
            )
            nc.gpsimd.dma_start(out=bv_bc, in_=bv_ap)

            xres_sb = persist.tile([P, NQT, GD], bf16, tag="xres_sb")
            xres_r = xres_d.rearrange("(t p) c -> p t c", p=P)

            def gated_dma(engine, dst, src):
                # DVE write to the destination's first element forces the
                # DMA to wait for the marker's queue position
                nc.vector.memset(dst[(slice(None),) + (0,) * (len(dst.shape) - 2)
                                     + (slice(0, 1),)], 0.0)
                engine.dma_start(dst, src)

            x23_gate = [
                lambda: gated_dma(nc.sync, x_sb[:, 2], x_r[:, 2]),
                lambda: gated_dma(nc.sync, x_sb[:, 3], x_r[:, 3]),
            ]
            xres_gate = [
                lambda: gated_dma(nc.sync, xres_sb[:, 0:8, :], xres_r[:, 0:8, :]),
                lambda: gated_dma(nc.sync, xres_sb[:, 8:16, :], xres_r[:, 8:16, :]),
            ]

            identity = persist.tile([P, P], bf16, tag="identity")
            make_identity(nc, identity)
            zero_sb = persist.tile([P, 512], bf16, tag="zero_sb")
            nc.vector.memset(zero_sb, 0.0)

            # pre-warm the PE while the first DMAs land: junk matmuls
            # take the HAM clock gate to 8/8 before the real work starts
            # (sized to span the DMA wait so no idle window re-throttles)
            warm_ps = psum.tile([HD, 512], f32, tag="flex", name="warm_ps")
            for _ in range(22):
                nc.tensor.matmul(
                    warm_ps, lhsT=zero_sb[:, 0:HD], rhs=zero_sb,
                    start=True, stop=True)

            # x tokens 512-1023 is needed a few us after the first
            # projections: release its DMA from the gpsimd queue behind a
            # ~2us memset shim so wa/x0/wv get the startup bandwidth alone
            v_sb = persist.tile([P, KT, GH * VC], av_dt, tag="v_sb")
            nc.sync.dma_start(
                wb_sb, wb_d.rearrange("p (dt s) -> p dt s", dt=DT))
            nc.gpsimd.memset(v_sb[:, 0:8, :], 0.0)
            nc.gpsimd.dma_start(x_sb[:, 1], x_r[:, 1])    # x 512-1023

            qT_sb = persist.tile([P, 2, S], bf16, tag="qT_sb")
            if USE_ROWTILE:
                kT_sb = persist.tile([P, 2, S], bf16, tag="kT_sb")

                def kzero_unit(m, c):
                    pass
            else:
                kT_sb = persist.tile([P, GH, S], bf16, tag="kT_sb")
                kq = kT_sb.rearrange("p (m two) s -> p m two s", two=2)

                def kzero_unit(m, c):
                    # zero the dead partition halves of head-pair m's K slots
                    # for 512 tokens -- emitted just-in-time so the bulk
                    # memset doesn't sit ahead of the first evacuations in
                    # the DVE queue
                    sl = slice(c * 512, (c + 1) * 512)
                    nc.vector.memset(kq[HD:, m, 0, sl], 0.0)
                    nc.vector.memset(kq[:HD, m, 1, sl], 0.0)

            def ones_unit():
                nc.vector.memset(
                    v_sb.rearrange(
                        "p t (h c) -> p t h c", c=VC)[:, :, :, HD:HD + 1],
                    1.0)
            out_sb = persist.tile([P, NQT, GD], f32, tag="out_sb")

            # pre-observe DMA'd constants on DVE so downstream DVE ops don't
            # carry a DMA wait alongside a PE wait
            sink = persist.tile([P, 4], f32, tag="sink")
            nc.vector.tensor_copy(sink[:, 0:1], bv_bc[:, 0:1])
            nc.vector.tensor_copy(sink[:, 1:2], bq_sb[:, 0:1])
            xres_sink = [
                lambda: nc.vector.tensor_copy(sink[:, 2:3], xres_sb[:, 0, 0:1]),
                lambda: nc.vector.tensor_copy(sink[:, 3:4], xres_sb[:, 8, 0:1]),
            ]

            # ---- projections (fp8 DoubleRow: 256-deep contraction/MM) ----
            def proj_qk_unit(which, m, c):
                w = wq_sb if which == "q" else wk_sb
                ps = psum.tile([P, 512], f32, tag="flex", name="ps_proj")
                xc = xt(c)
                for dp in range(DT // 2):
                    nc.tensor.matmul(
                        ps, lhsT=w[:, 2 * dp:2 * dp + 2, m * P:(m + 1) * P],
                        rhs=xc[:, 2 * dp:2 * dp + 2, :],
                        start=(dp == 0), stop=(dp == DT // 2 - 1),
                        perf_mode=DR,
                    )
                sl = slice(c * 512, (c + 1) * 512)
                if which == "q":
                    nc.vector.tensor_scalar_add(
                        qT_sb[:, m, sl], ps, bq_sb[:, m:m + 1])
                elif USE_ROWTILE:
                    nc.vector.tensor_copy(kT_sb[:, m, sl], ps)
                else:
                    nc.vector.tensor_copy(kT_sb[:HD, 2 * m, sl], ps[:HD])
                    nc.vector.tensor_copy(kT_sb[HD:, 2 * m + 1, sl], ps[HD:])

            def v_unit(tt):
                ps = psum.tile([P, GD], f32, tag="flex", name="ps_v")
                xc = xt(tt // 4)
                o = (tt % 4) * P
                for dp in range(DT // 2):
                    nc.tensor.matmul(
                        ps, lhsT=xc[:, 2 * dp:2 * dp + 2, o:o + P],
                        rhs=wv_sb[:, 2 * dp:2 * dp + 2, :],
                        start=(dp == 0), stop=(dp == DT // 2 - 1),
                        perf_mode=DR,
                    )
                nc.vector.tensor_tensor(
                    v_sb[:, tt, :].rearrange(
                        "p (h c) -> p h c", c=VC)[:, :, :HD],
                    ps.rearrange("p (h c) -> p h c", c=HD),
                    bv_bc.rearrange("p (h c) -> p h c", c=HD),
                    ADD)

            # ---- attention ----
            def av_mm_unit(m, qb, par, qh, exps, t0=0, t1=None, pso=None):
                h = 2 * m + par
                if pso is None:
                    pso = psum.tile([65, 512], f32, tag="flex", name="pso")
                ex = exps.rearrange(
                    "p (qh kt par q) -> p qh kt par q", kt=KT, qh=2, par=2)
                n = KT // 2 if USE_FP8AV else KT
                rng = range(n) if t1 is None else range(t0, t1)
                for t in rng:
                    first = t == 0
                    last = t == n - 1
                    if USE_FP8AV:
                        nc.tensor.matmul(
                            pso,
                            lhsT=v_sb[:, 2 * t:2 * t + 2, h * VC:h * VC + 65],
                            rhs=ex[:, qh, 2 * t:2 * t + 2, par, :],
                            start=first, stop=last, perf_mode=DR)
                    else:
                        nc.tensor.matmul(
                            pso,
                            lhsT=v_sb[:, t, h * VC:h * VC + 65],
                            rhs=ex[:, qh, t, par, :],
                            start=first, stop=last)
                return pso

            def av_out_unit(m, qb, par, qh, pso, dma_qt, act_copy=False):
                h = 2 * m + par
                oT = work.tile([65, 512], bf16, tag="oT", name="oT")
                if act_copy:
                    # final-block output chain: the exp stream is done, so
                    # the idle ACT engine does the PSUM evacuation instead
                    # of the busy DVE
                    nc.scalar.copy(oT, pso)
                else:
                    nc.vector.tensor_copy(oT, pso)
                pst = psum.tile([P, 4 * 65], f32, tag="flex", name="pst")
                for i in range(4):
                    nc.tensor.matmul(
                        pst[:, i * 65:(i + 1) * 65],
                        lhsT=oT[:, i * 128:(i + 1) * 128],
                        rhs=identity[:65, :65],
                        start=True, stop=True)
                r = work.tile([P, 4], f32, tag="r", name="r")
                nc.vector.reciprocal(
                    r.rearrange("p (i c) -> p i c", c=1),
                    pst.rearrange("p (i c) -> p i c", c=65)[:, :, 64:65])
                # out-DMAs spread across engine queues (issuance costs
                # ~0.6us per DMA on one sequencer); scalar only at the very
                # tail when the exp stream is finished
                engs = [nc.sync, nc.gpsimd, nc.sync, nc.gpsimd]
                if m == 1 and qb == 1 and qh == 1:
                    engs = [nc.sync, nc.gpsimd, nc.scalar, nc.sync]
                for i in range(4):
                    qt = qb * 8 + qh * 4 + i
                    nc.vector.scalar_tensor_tensor(
                        out=out_sb[:, qt, h * HD:(h + 1) * HD],
                        in0=pst[:, i * 65:i * 65 + 64],
                        scalar=r[:, i:i + 1],
                        in1=xres_sb[:, qt, h * HD:(h + 1) * HD],
                        op0=MULT, op1=ADD)
                    if dma_qt:
                        engs[i].dma_start(
                            out_d[qt * P:(qt + 1) * P, :], out_sb[:, qt, :])

            def block_scores(m, qb, exps, fill, pre):
                """scores^T + exp for head pair m, 1024-wide q block qb.

                unit u = (qh, kt, par):   exps flat = (qh kt par q)
                fill[i]: emitted after the i-th ACT chunk (None = skip).
                pre[u]: callables emitted before unit u's matmul.
                """
                fi = 0
                ci = 0
                ring_t = None
                chunk_start = 0
                for u in range(NU):
                    qh, kt, par = u // 32, (u // 2) % KT, u % 2
                    for fn in pre.get(u, ()):
                        fn()
                    # runt chunk (1 unit) first: the next block's exp stream
                    # restarts one short matmul after the boundary
                    j = 0 if u == 0 else (u - 1) % 3
                    if j == 0:
                        ring_t = psum.tile(
                            [P, 1536], f32, tag="ring", name="ring")
                        chunk_start = u
                    q0 = qb * 1024 + qh * 512
                    if USE_ROWTILE:
                        rows = slice(par * HD, (par + 1) * HD)
                        nc.tensor.matmul(
                            ring_t[:, j * 512:(j + 1) * 512],
                            lhsT=kT_sb[rows, m, kt * P:(kt + 1) * P],
                            rhs=qT_sb[rows, m, q0:q0 + 512],
                            start=True, stop=True,
                            tile_position=(par * HD, 0))
                    else:
                        nc.tensor.matmul(
                            ring_t[:, j * 512:(j + 1) * 512],
                            lhsT=kT_sb[:, 2 * m + par, kt * P:(kt + 1) * P],
                            rhs=qT_sb[:, m, q0:q0 + 512],
                            start=True, stop=True)
                    if j == 2 or u == 0 or u == NU - 1:
                        w = (u - chunk_start + 1) * 512
                        dst = exps[:, chunk_start * 512:chunk_start * 512 + w]
                        if USE_FP8AV and ci % 4 == 2:
                            # fast-exp on DVE: uint8(round(s*log2e + 8*bias))
                            # IS fp8e4 of exp2(s*log2e) up to the linear-
                            # mantissa approx (<6%, and the ones-column
                            # denominator sums the same stored values).
                            # Offloads ~25% of exp from the saturated ACT.
                            nc.vector.tensor_scalar(
                                out=dst.bitcast(mybir.dt.uint8),
                                in0=ring_t[:, :w],
                                scalar1=0.125 * 1.4426950408889634,
                                scalar2=56.0,
                                op0=MULT, op1=ADD)
                        else:
                            nc.scalar.activation(
                                out=dst, in_=ring_t[:, :w],
                                func=EXP, scale=0.125)
                        ci += 1
                        if fi < len(fill):
                            if fill[fi] is not None:
                                fill[fi]()
                            fi += 1
                for fn in fill[fi:]:
                    if fn is not None:
                        fn()

            blocks = [(0, 0), (0, 1), (1, 0), (1, 1)]
            exps_t = {}

            def av_fill(b, qhs=(0, 1), mm_first=False):
                # AV matmuls split into two halves so no single fill item
                # exceeds the per-ACT-chunk PE slack (~0.8us)
                m, qb = blocks[b]
                state = {}
                items = []
                n = KT // 2 if USE_FP8AV else KT
                for qh in qhs:
                    pair = []
                    for par in (0, 1):
                        def mk_mm_a(par=par, qh=qh):
                            state[(par, qh)] = av_mm_unit(
                                m, qb, par, qh, exps_t[b], 0, n // 2)

                        def mk_mm_b(par=par, qh=qh):
                            av_mm_unit(m, qb, par, qh, exps_t[b], n // 2, n,
                                       pso=state[(par, qh)])

                        def mk_out(par=par, qh=qh):
                            av_out_unit(m, qb, par, qh, state[(par, qh)],
                                        dma_qt=(m == 1 and par == 1))
                        pair += [mk_mm_a, mk_mm_b, mk_out]
                    if mm_first:
                        # both AV matmul groups back-to-back, then the two
                        # output chains (shorter critical path at the tail)
                        pair = [pair[0], pair[1], pair[3], pair[4],
                                pair[2], pair[5]]
                    items += pair
                return items

            def av_stream(b, qh):
                """Final-block AV: accumulate in quarters as the exps land,
                so only the last quarter and the output chain trail the
                exp stream."""
                m, qb = blocks[b]
                n = KT // 2 if USE_FP8AV else KT
                state = {}
                items = []
                for j in range(4):
                    for par in (0, 1):
                        def mk(j=j, par=par):
                            t0, t1 = j * n // 4, (j + 1) * n // 4
                            state[par] = av_mm_unit(
                                m, qb, par, qh, exps_t[b], t0, t1,
                                pso=state.get(par))
                        items.append(mk)
                for par in (0, 1):
                    def mk_out(par=par):
                        av_out_unit(m, qb, par, qh, state[par],
                                    dma_qt=(m == 1 and par == 1))
                    items.append(mk_out)
                return items

            def spread(items, slots=22):
                out = []
                for i, it in enumerate(items):
                    out.append(it)
                    if i % 3 == 2 and len(out) + (len(items) - i - 1) < slots:
                        out.append(None)
                return out

            def interleave(a, b):
                out = []
                for x, y in zip(a, b):
                    out += [x, y]
                out += a[len(b):] + b[len(a):]
                return out

            # prologue: only what the first score matmuls need
            kzero_unit(0, 0)
            proj_qk_unit("k", 0, 0)
            proj_qk_unit("q", 0, 0)

            for b, (m, qb) in enumerate(blocks):
                exps_t[b] = expp.tile(
                    [P, NU * 512], av_dt, tag="exps", name="exps")
                pre = {}
                if b == 0:
                    # K M0 chunks staged at the kt boundaries that need them;
                    # Q M0 chunk 1 before the qh=1 half; gated DMAs released
                    # once the startup-critical transfers are done
                    # pre keys sit on chunk starts (u = 1+3k) so a staged
                    # projection never lands inside a chunk's matmul group
                    pre = {
                        4: [lambda: proj_qk_unit("k", 0, 1),
                            x23_gate[0], x23_gate[1], ones_unit],
                        7: [lambda: kzero_unit(0, 1)],
                        10: [lambda: proj_qk_unit("k", 0, 2)],
                        16: [lambda: kzero_unit(0, 2),
                             xres_gate[0], xres_sink[0]],
                        19: [lambda: proj_qk_unit("k", 0, 3)],
                        22: [lambda: kzero_unit(0, 3),
                             xres_gate[1], xres_sink[1]],
                        31: [lambda: proj_qk_unit("q", 0, 1)],
                    }
                    # V units, then Q chunk 2 (needed by block1's qh=0), then
                    # block0's own qh=0 AV work -- its exps are complete by
                    # chunk 11, and block1 has no room for it (scores + M1
                    # projections + its own AV would oversubscribe the PE)
                    fill = [
                        (lambda tt=tt: v_unit(tt)) for tt in range(KT)
                    ] + [
                        lambda: proj_qk_unit("q", 0, 2),
                    ] + av_fill(0, qhs=(0,))
                elif b == 1:
                    def qc3_kz(_=None):
                        proj_qk_unit("q", 0, 3)
                        kzero_unit(1, 2)
                        kzero_unit(1, 3)

                    def kz01(_=None):
                        kzero_unit(1, 0)
                        kzero_unit(1, 1)
                    proj1 = [
                        (lambda c=c: proj_qk_unit("k", 1, c)) for c in range(4)
                    ] + [
                        (lambda c=c: proj_qk_unit("q", 1, c)) for c in range(4)
                    ]
                    fill = spread([kz01, qc3_kz] + interleave(
                        proj1, av_fill(0, qhs=(1,))))
                elif b == 2:
                    fill = spread(av_fill(1))
                else:
                    # previous block's AV, this block's qh=0 AV once its
                    # exps complete (chunk 11), then the qh=1 AV streamed
                    # in quarters behind the exp stream
                    st = av_stream(3, 1)
                    fill = av_fill(2) + av_fill(3, qhs=(0,)) + st[:4]
                    tail_items = st[4:]
                block_scores(m, qb, exps_t[b], fill, pre)

            for it in tail_items:
                it()

    nc.finalize()
    return nc


def _get_nc():
    if "nc" not in _CACHE:
        _CACHE["nc"] = _build_nc()
    return _CACHE["nc"]


def _swizzle(a):
    """[1024, n] -> [128, 8, n] with row d at (d % 128, d // 128)."""
    return np.ascontiguousarray(
        a.reshape(DT, P, a.shape[1]).transpose(1, 0, 2))


FP8 = ml_dtypes.float8_e4m3  # IEEE e4m3 (bias 7, max 240) = TRN FP8_EXP4


def kernel(x, Wq, bq, Wk, bk, Wv, bv):
    global LAST_RESULTS
    from concourse.bass_utils import run_bass_kernel_spmd

    x = np.asarray(x, dtype=np.float32)
    Wq, Wk, Wv = (np.asarray(a, dtype=np.float32) for a in (Wq, Wk, Wv))
    bq, bv = (np.asarray(a, dtype=np.float32) for a in (bq, bv))

    xTs = [np.ascontiguousarray(x[b].T) for b in range(B)]
    xt8 = []
    for b in range(B):
        xc = np.empty((P, 4, DT, 512), dtype=FP8)
        for ch in range(4):
            xc[:, ch] = _swizzle(
                xTs[b][:, ch * 512:(ch + 1) * 512]).astype(FP8)
        xt8.append(np.ascontiguousarray(xc.reshape(P, 4 * DT * 512)))
    in_maps = []
    for c in range(NCORES):
        b, g = c // 4, c % 4
        cols = slice(GD * g, GD * (g + 1))
        wa = np.empty((P, DT, 512), dtype=FP8)
        wa[:, :, 0:GD] = _swizzle(Wq[:, cols]).astype(FP8)
        wa[:, :, GD:2 * GD] = _swizzle(Wk[:, cols]).astype(FP8)
        in_maps.append({
            "wa": np.ascontiguousarray(wa.reshape(P, DT * 512)),
            "wb": np.ascontiguousarray(
                _swizzle(Wv[:, cols]).astype(FP8).reshape(P, DT * GD)),
            "xt": xt8[b],
            "bq": np.ascontiguousarray(bq[cols]),
            "bv": np.ascontiguousarray(bv[cols]),
            "xres": np.ascontiguousarray(
                x[b][:, cols].astype(ml_dtypes.bfloat16)),
        })

    nc = _get_nc()
    res = run_bass_kernel_spmd(
        nc, in_maps, core_ids=list(range(NCORES)), trace=TRACE,
    )
    LAST_RESULTS = res

    full = np.empty((B, S, D), dtype=np.float32)
    for c in range(NCORES):
        b, g = c // 4, c % 4
        full[b, :, GD * g:GD * (g + 1)] = res.results[c]["out"]
    return full
